# revision 1
# baseline (speedup 1.0000x reference)
"""Trainium2 Bass kernel for GainesEdgeDetect (single stochastic bit-cycle).

The reference module hardcodes sel=0 (first Sobol draw), so the MUXes
statically select their first operand and the output reduces to a pointwise
function of only inp_Pr_i_j (x) and cnt_x (c):

    A    = c + 2*x            (counter update, pre-clip)
    mask = (A - 1) < 8        (clip to [0,15] cannot change this comparison)
    out  = mask ? (1 - x) : x

Bit-exact mapping onto the engines, per [128 x CHUNK] tile:

    V: A    = (x mult 2.0) add c          scalar_tensor_tensor, 1x fp32
    V: mask = (A sub 1.0) is_lt 8.0       tensor_scalar (2 fused ops), 2x fp32
    S: u    = Copy(-1.0 * x + 1.0)        ScalarE activation
    V: x    = where(mask, u, x)           copy_predicated, in place
    then DMA x back out.

Sharding: pointwise over 16M elements; each of the 8 cores takes a
contiguous 1/8th (2M elements) viewed as [128 partitions x 16384], streamed
through SBUF in [128 x CHUNK] chunks, triple buffered. No cross-core
communication.
"""

import sys

for _p in ("/opt/trn_rl_repo", "/root/.axon_site/_ro/trn_rl_repo"):
    if _p not in sys.path:
        sys.path.append(_p)

import numpy as np

import concourse.bacc as bacc
import concourse.bass as bass
import concourse.mybir as mybir
from concourse.bass_utils import run_bass_kernel_spmd
from concourse.tile import TileContext

N_CORES = 8
FULL_SHAPE = (16, 1024, 1024)
TOTAL = FULL_SHAPE[0] * FULL_SHAPE[1] * FULL_SHAPE[2]
PER_CORE = TOTAL // N_CORES  # 2M elements
P = 128  # SBUF partitions
CHUNK = 2048

# Set by test harness to capture an NTFF profile of the run.
TRACE = False
TMPDIR = None
LAST_RESULTS = None


def build_kernel(fd: int, chunk: int) -> bass.Bass:
    """Per-core program: x[P, fd], cnt[P, fd] -> out[P, fd]."""
    assert fd % chunk == 0
    # Bacc (not plain Bass): its generate_event_semaphores pass splits
    # multi-sem waits into EventSemaphore instructions — TRN2 TPB compute
    # instructions can carry at most one sync-wait command.
    nc = bacc.Bacc()
    dt = mybir.dt.float32
    x = nc.declare_dram_parameter("x", [P, fd], dt, isOutput=False)
    cnt = nc.declare_dram_parameter("cnt", [P, fd], dt, isOutput=False)
    out = nc.declare_dram_parameter("out", [P, fd], dt, isOutput=True)

    with TileContext(nc) as tc:
        with (
            tc.tile_pool(name="xp", bufs=3) as xp,
            tc.tile_pool(name="cp", bufs=3) as cp,
            tc.tile_pool(name="ap", bufs=3) as ap,
            tc.tile_pool(name="mp", bufs=3) as mp,
            tc.tile_pool(name="up", bufs=3) as up,
        ):
            for i in range(fd // chunk):
                sl = bass.ts(i, chunk)
                xt = xp.tile([P, chunk], dt)
                ct = cp.tile([P, chunk], dt)
                nc.sync.dma_start(xt[:], x[:, sl])
                nc.sync.dma_start(ct[:], cnt[:, sl])
                at = ap.tile([P, chunk], dt)
                # A = 2x + cnt
                nc.vector.scalar_tensor_tensor(
                    at[:], xt[:], 2.0, ct[:],
                    mybir.AluOpType.mult, mybir.AluOpType.add,
                )
                # mask = (A - 1) < 8, as int32 (CopyPredicated wants an
                # integer mask dtype; 32-bit keeps the 2x DVE perf mode)
                mt = mp.tile([P, chunk], mybir.dt.int32)
                nc.vector.tensor_scalar(
                    mt[:], at[:], 1.0, 8.0,
                    mybir.AluOpType.subtract, mybir.AluOpType.is_lt,
                )
                # u = 1 - x on the scalar engine
                ut = up.tile([P, chunk], dt)
                nc.scalar.activation(
                    ut[:], xt[:], mybir.ActivationFunctionType.Copy,
                    bias=1.0, scale=-1.0,
                )
                # x = where(mask, 1-x, x), in place; then store
                nc.vector.copy_predicated(xt[:], mt[:], ut[:])
                nc.sync.dma_start(out[:, sl], xt[:])
    # Run Bacc's compile passes (event-sem splitting, register allocation).
    nc.finalize()
    return nc


_NC_CACHE: dict[tuple[int, int], bass.Bass] = {}


def _get_nc(fd: int, chunk: int) -> bass.Bass:
    key = (fd, chunk)
    if key not in _NC_CACHE:
        _NC_CACHE[key] = build_kernel(fd, chunk)
    return _NC_CACHE[key]


def kernel(**inputs: np.ndarray) -> np.ndarray:
    global LAST_RESULTS
    x_full = np.ascontiguousarray(inputs["inp_Pr_i_j"], dtype=np.float32)
    c_full = np.ascontiguousarray(inputs["cnt_x"], dtype=np.float32)
    assert x_full.shape == FULL_SHAPE and c_full.shape == FULL_SHAPE

    fd = PER_CORE // P  # 16384
    nc = _get_nc(fd, CHUNK)

    xs = x_full.reshape(N_CORES, P, fd)
    cs = c_full.reshape(N_CORES, P, fd)
    in_maps = [{"x": xs[c], "cnt": cs[c]} for c in range(N_CORES)]
    res = run_bass_kernel_spmd(
        nc, in_maps, list(range(N_CORES)), trace=TRACE, tmpdir=TMPDIR
    )
    LAST_RESULTS = res
    out = np.stack([res.results[c]["out"] for c in range(N_CORES)], axis=0)
    return np.ascontiguousarray(out.reshape(FULL_SHAPE).astype(np.float32))



# revision 5
# speedup vs baseline: 1.7509x; 1.7509x over previous
"""Trainium2 Bass kernel for GainesEdgeDetect (single stochastic bit-cycle).

The reference module hardcodes sel=0 (first Sobol draw), so the MUXes
statically select their first operand and the output reduces to a pointwise
function of only inp_Pr_i_j (x) and cnt_x (c):

    A    = c + 2*x            (counter update, pre-clip)
    mask = (A - 1) < 8        (clip to [0,15] cannot change this comparison)
    out  = mask ? (1 - x) : x

Fast path (x is a 0/1 bitstream, c an integer counter in [0, 15] — the
module's actual operating domain): out == x XOR (2x + c < 9).  With
U = 16x + c (ranges of the two x-cases don't overlap), the truth table
collapses to a single band test:

    out = 1  iff  U <= 8  or  U >= 23   ==   |U - 15.5| > 7.0

which maps onto one DVE op + one ACT op + one DVE op per tile:

    V: U = (x mult 16) add c     scalar_tensor_tensor, bf16 (2x mode)
    S: A = Abs(U - 15.5)         ScalarE activation, bf16
    V: o = A is_gt 7.0           tensor_scalar, bf16 in (4x/2x mode)

HBM traffic is cut 4x by keeping fp8_e4m3 on the wire (exact for 0/1 bits
and integer counters <= 15): inputs are cast fp8->bf16 during the SWDGE
load, the output is written back as fp8.  6 MiB/core instead of 24 MiB.

Inputs that fail the domain check fall back to an exact f32 kernel.

Sharding: pointwise over 16M elements; each of the 8 cores takes a
contiguous 1/8th (2M elements) viewed as [128 partitions x 16384].
"""

import sys

for _p in ("/opt/trn_rl_repo", "/root/.axon_site/_ro/trn_rl_repo"):
    if _p not in sys.path:
        sys.path.append(_p)

import ml_dtypes
import numpy as np

import concourse.bacc as bacc
import concourse.bass as bass
import concourse.mybir as mybir
from concourse.bass_utils import run_bass_kernel_spmd
from concourse.tile import TileContext

N_CORES = 8
FULL_SHAPE = (16, 1024, 1024)
TOTAL = FULL_SHAPE[0] * FULL_SHAPE[1] * FULL_SHAPE[2]
PER_CORE = TOTAL // N_CORES  # 2M elements
P = 128  # SBUF partitions
CHUNK = 4096  # fast-path tile free-dim
CHUNK_F32 = 2048  # fallback tile free-dim

FP8 = ml_dtypes.float8_e4m3  # == mybir.dt.float8e4 wire format

# Set by test harness to capture an NTFF profile of the run.
TRACE = False
TMPDIR = None
LAST_RESULTS = None


def build_kernel_fp8(fd: int, chunk: int) -> bass.Bass:
    """Fast path: x[P, fd] fp8 bits, cnt[P, fd] fp8 ints -> out[P, fd] fp8."""
    assert fd % chunk == 0
    nc = bacc.Bacc()
    f8 = mybir.dt.float8e4
    bf = mybir.dt.bfloat16
    x = nc.declare_dram_parameter("x", [P, fd], f8, isOutput=False)
    cnt = nc.declare_dram_parameter("cnt", [P, fd], f8, isOutput=False)
    out = nc.declare_dram_parameter("out", [P, fd], f8, isOutput=True)

    # ACT converts a float bias into a const AP; -15.5 isn't in the default
    # registry, so register it the same way Bass.__init__ does.
    bias_t = nc.alloc_sbuf_tensor("const-float32--15.5", [128, 1], mybir.dt.float32)
    nc.gpsimd.memset(bias_t.ap(), -15.5)
    nc.const_aps.aps[(mybir.dt.float32, -15.5)] = bias_t.ap()
    nc.all_engine_barrier()

    with TileContext(nc) as tc:
        with (
            tc.tile_pool(name="xp", bufs=3) as xp,
            tc.tile_pool(name="cp", bufs=3) as cp,
            tc.tile_pool(name="op", bufs=3) as op,
        ):
            for i in range(fd // chunk):
                sl = bass.ts(i, chunk)
                xt = xp.tile([P, chunk], bf)
                ct = cp.tile([P, chunk], bf)
                # SWDGE loads with fp8 -> bf16 upcast in the DMA datapath.
                nc.gpsimd.dma_start(xt[:], x[:, sl])
                nc.gpsimd.dma_start(ct[:], cnt[:, sl])
                # U = 16x + c, in place over c (bf16 keeps DVE in 2x mode)
                nc.vector.scalar_tensor_tensor(
                    ct[:], xt[:], 16.0, ct[:],
                    mybir.AluOpType.mult, mybir.AluOpType.add,
                )
                # A = |U - 15.5| on the scalar engine, in place
                nc.scalar.activation(
                    ct[:], ct[:], mybir.ActivationFunctionType.Abs,
                    bias=-15.5, scale=1.0,
                )
                # out = (A > 7.0), written directly as fp8 for the store
                ot = op.tile([P, chunk], f8)
                nc.vector.tensor_scalar(
                    ot[:], ct[:], 7.0, None, mybir.AluOpType.is_gt,
                )
                nc.sync.dma_start(out[:, sl], ot[:])
    nc.finalize()
    return nc


def build_kernel_f32(fd: int, chunk: int) -> bass.Bass:
    """Fallback, exact for any f32 inputs: x[P, fd], cnt[P, fd] -> out[P, fd]."""
    assert fd % chunk == 0
    # Bacc (not plain Bass): its generate_event_semaphores pass splits
    # multi-sem waits into EventSemaphore instructions — TRN2 TPB compute
    # instructions can carry at most one sync-wait command.
    nc = bacc.Bacc()
    dt = mybir.dt.float32
    x = nc.declare_dram_parameter("x", [P, fd], dt, isOutput=False)
    cnt = nc.declare_dram_parameter("cnt", [P, fd], dt, isOutput=False)
    out = nc.declare_dram_parameter("out", [P, fd], dt, isOutput=True)

    with TileContext(nc) as tc:
        with (
            tc.tile_pool(name="xp", bufs=3) as xp,
            tc.tile_pool(name="cp", bufs=3) as cp,
            tc.tile_pool(name="ap", bufs=3) as ap,
            tc.tile_pool(name="mp", bufs=3) as mp,
            tc.tile_pool(name="up", bufs=3) as up,
        ):
            for i in range(fd // chunk):
                sl = bass.ts(i, chunk)
                xt = xp.tile([P, chunk], dt)
                ct = cp.tile([P, chunk], dt)
                nc.sync.dma_start(xt[:], x[:, sl])
                nc.sync.dma_start(ct[:], cnt[:, sl])
                at = ap.tile([P, chunk], dt)
                # A = 2x + cnt
                nc.vector.scalar_tensor_tensor(
                    at[:], xt[:], 2.0, ct[:],
                    mybir.AluOpType.mult, mybir.AluOpType.add,
                )
                # mask = (A - 1) < 8, as int32 (CopyPredicated wants an
                # integer mask dtype; 32-bit keeps the 2x DVE perf mode)
                mt = mp.tile([P, chunk], mybir.dt.int32)
                nc.vector.tensor_scalar(
                    mt[:], at[:], 1.0, 8.0,
                    mybir.AluOpType.subtract, mybir.AluOpType.is_lt,
                )
                # u = 1 - x on the scalar engine
                ut = up.tile([P, chunk], dt)
                nc.scalar.activation(
                    ut[:], xt[:], mybir.ActivationFunctionType.Copy,
                    bias=1.0, scale=-1.0,
                )
                # x = where(mask, 1-x, x), in place; then store
                nc.vector.copy_predicated(xt[:], mt[:], ut[:])
                nc.sync.dma_start(out[:, sl], xt[:])
    nc.finalize()
    return nc


_NC_CACHE: dict[tuple, bass.Bass] = {}


def _get_nc(kind: str, fd: int, chunk: int) -> bass.Bass:
    key = (kind, fd, chunk)
    if key not in _NC_CACHE:
        build = build_kernel_fp8 if kind == "fp8" else build_kernel_f32
        _NC_CACHE[key] = build(fd, chunk)
    return _NC_CACHE[key]


def _fast_path_ok(x: np.ndarray, c: np.ndarray) -> bool:
    # Fast path needs x to be 0/1 bits and c an integer counter in [0, 15]
    # (the FSUAbs operating domain); both are then exact in fp8_e4m3.
    if not np.all((x == 0.0) | (x == 1.0)):
        return False
    if not (c.min() >= 0.0 and c.max() <= 15.0):
        return False
    return bool(np.all(c == np.trunc(c)))


def kernel(**inputs: np.ndarray) -> np.ndarray:
    global LAST_RESULTS
    x_full = np.ascontiguousarray(inputs["inp_Pr_i_j"], dtype=np.float32)
    c_full = np.ascontiguousarray(inputs["cnt_x"], dtype=np.float32)
    assert x_full.shape == FULL_SHAPE and c_full.shape == FULL_SHAPE

    fd = PER_CORE // P  # 16384
    fast = _fast_path_ok(x_full, c_full)
    if fast:
        nc = _get_nc("fp8", fd, CHUNK)
        xs = x_full.astype(FP8).reshape(N_CORES, P, fd)
        cs = c_full.astype(FP8).reshape(N_CORES, P, fd)
    else:
        nc = _get_nc("f32", fd, CHUNK_F32)
        xs = x_full.reshape(N_CORES, P, fd)
        cs = c_full.reshape(N_CORES, P, fd)

    in_maps = [{"x": xs[c], "cnt": cs[c]} for c in range(N_CORES)]
    res = run_bass_kernel_spmd(
        nc, in_maps, list(range(N_CORES)), trace=TRACE, tmpdir=TMPDIR
    )
    LAST_RESULTS = res
    out = np.stack([res.results[c]["out"] for c in range(N_CORES)], axis=0)
    return np.ascontiguousarray(
        out.reshape(FULL_SHAPE).astype(np.float32)
    )


# revision 9
# speedup vs baseline: 2.0671x; 1.1806x over previous
"""Trainium2 Bass kernel for GainesEdgeDetect (single stochastic bit-cycle).

The reference module hardcodes sel=0 (first Sobol draw), so the MUXes
statically select their first operand and the output reduces to a pointwise
function of only inp_Pr_i_j (x) and cnt_x (c):

    A    = c + 2*x            (counter update, pre-clip)
    mask = (A - 1) < 8        (clip to [0,15] cannot change this comparison)
    out  = mask ? (1 - x) : x

Fast path (x is a 0/1 bitstream, c an integer counter in [0, 15] — the
module's actual operating domain): out == x XOR (2x + c < 9).  With
U = 16x + c (ranges of the two x-cases don't overlap), the truth table
collapses to a single band test:

    out = 1  iff  U <= 8  or  U >= 23   ==   |U - 15.5| > 7.0

Engine mapping, chosen from measured constraints (DVE two-tensor ops on
8-bit operands run at 1x = ~17.6us/core alone; DMA-casting fp8->bf16 paces
the SBUF fabric at the expanded byte count):

    PE : U = W.T @ moving           fp8 matmul; W block-diag carries the
                                    16/1 coefficients, two 64-row halves
    ACT: A = Abs(U - 15.5)          straight from PSUM, writes bf16
    DVE: o = (A is_gt 7.0)          tensor_scalar, fp8 out (2x mode)

HBM traffic is cut 4x by keeping fp8_e4m3 on the wire (exact for 0/1 bits
and integer counters <= 15): 6 MiB/core instead of 24 MiB.  All DMAs are
plain HWDGE.  The x/c planes are shipped interleaved in two half-stacks
(a1 = [x rows 0:64; c rows 0:64], a2 = same for rows 64:128) so each
matmul reads one [128, N] tile and produces one 64-row half of U.

Inputs that fail the domain check fall back to an exact f32 kernel.

Sharding: pointwise over 16M elements; each of the 8 cores takes a
contiguous 1/8th (2M elements) viewed as [128 partitions x 16384].
"""

import sys

for _p in ("/opt/trn_rl_repo", "/root/.axon_site/_ro/trn_rl_repo"):
    if _p not in sys.path:
        sys.path.append(_p)

import ml_dtypes
import numpy as np

import concourse.bacc as bacc
import concourse.bass as bass
import concourse.mybir as mybir
from concourse.bass_utils import run_bass_kernel_spmd
from concourse.tile import TileContext

N_CORES = 8
FULL_SHAPE = (16, 1024, 1024)
TOTAL = FULL_SHAPE[0] * FULL_SHAPE[1] * FULL_SHAPE[2]
PER_CORE = TOTAL // N_CORES  # 2M elements
P = 128  # SBUF partitions
H = P // 2  # half-stack rows
CHUNK = 2048  # fast-path tile free-dim ([128, 2048] f32 psum = 4 banks)
BANK = 512  # one PSUM bank holds 512 f32 per partition
CHUNK_F32 = 2048  # fallback tile free-dim

FP8 = ml_dtypes.float8_e4m3  # == mybir.dt.float8e4 wire format

# Set by test harness to capture an NTFF profile of the run.
TRACE = False
TMPDIR = None
LAST_RESULTS = None


def build_kernel_pe(fd: int, chunk: int) -> bass.Bass:
    """Fast path: a1/a2 [P, fd] fp8 half-stacks -> out[P, fd] fp8."""
    assert fd % chunk == 0 and chunk % BANK == 0
    nc = bacc.Bacc()
    f8 = mybir.dt.float8e4
    bf = mybir.dt.bfloat16
    f32 = mybir.dt.float32
    a1 = nc.declare_dram_parameter("a1", [P, fd], f8, isOutput=False)
    a2 = nc.declare_dram_parameter("a2", [P, fd], f8, isOutput=False)
    w = nc.declare_dram_parameter("w", [P, H], f8, isOutput=False)
    out = nc.declare_dram_parameter("out", [P, fd], f8, isOutput=True)

    # ACT converts a float bias into a const AP; -15.5 isn't in the default
    # registry, so register it the same way Bass.__init__ does.
    bias_t = nc.alloc_sbuf_tensor("const-f32-m15p5", [P, 1], f32)
    nc.gpsimd.memset(bias_t.ap(), -15.5)
    nc.const_aps.aps[(f32, -15.5)] = bias_t.ap()
    nc.all_engine_barrier()

    with TileContext(nc) as tc:
        with (
            tc.tile_pool(name="wp", bufs=1) as wp,
            tc.tile_pool(name="r1p", bufs=3) as r1p,
            tc.tile_pool(name="r2p", bufs=3) as r2p,
            tc.tile_pool(name="ap", bufs=3) as ap,
            tc.tile_pool(name="op", bufs=3) as op,
            tc.tile_pool(name="pp", bufs=2, space="PSUM") as pp,
        ):
            wt = wp.tile([P, H], f8)
            nc.sync.dma_start(wt[:], w[:, :])
            for i in range(fd // chunk):
                sl = bass.ts(i, chunk)
                r1 = r1p.tile([P, chunk], f8)
                r2 = r2p.tile([P, chunk], f8)
                nc.sync.dma_start(r1[:], a1[:, sl])
                nc.sync.dma_start(r2[:], a2[:, sl])
                # U = 16x + c, one bank-sized matmul per 64-row half
                pt = pp.tile([P, chunk], f32)
                for b in range(chunk // BANK):
                    bs = bass.ts(b, BANK)
                    nc.tensor.matmul(pt[0:H, bs], wt[:], r1[:, bs])
                    nc.tensor.matmul(pt[H:P, bs], wt[:], r2[:, bs])
                # A = |U - 15.5|, PSUM -> SBUF on the scalar engine
                at = ap.tile([P, chunk], bf)
                nc.scalar.activation(
                    at[:], pt[:], mybir.ActivationFunctionType.Abs,
                    bias=-15.5, scale=1.0,
                )
                # out = (A > 7.0), written as fp8 for the store
                ot = op.tile([P, chunk], f8)
                nc.vector.tensor_scalar(
                    ot[:], at[:], 7.0, None, mybir.AluOpType.is_gt,
                )
                nc.sync.dma_start(out[:, sl], ot[:])
    nc.finalize()
    return nc


def build_kernel_f32(fd: int, chunk: int) -> bass.Bass:
    """Fallback, exact for any f32 inputs: x[P, fd], cnt[P, fd] -> out[P, fd]."""
    assert fd % chunk == 0
    # Bacc (not plain Bass): its generate_event_semaphores pass splits
    # multi-sem waits into EventSemaphore instructions — TRN2 TPB compute
    # instructions can carry at most one sync-wait command.
    nc = bacc.Bacc()
    dt = mybir.dt.float32
    x = nc.declare_dram_parameter("x", [P, fd], dt, isOutput=False)
    cnt = nc.declare_dram_parameter("cnt", [P, fd], dt, isOutput=False)
    out = nc.declare_dram_parameter("out", [P, fd], dt, isOutput=True)

    with TileContext(nc) as tc:
        with (
            tc.tile_pool(name="xp", bufs=3) as xp,
            tc.tile_pool(name="cp", bufs=3) as cp,
            tc.tile_pool(name="ap", bufs=3) as ap,
            tc.tile_pool(name="mp", bufs=3) as mp,
            tc.tile_pool(name="up", bufs=3) as up,
        ):
            for i in range(fd // chunk):
                sl = bass.ts(i, chunk)
                xt = xp.tile([P, chunk], dt)
                ct = cp.tile([P, chunk], dt)
                nc.sync.dma_start(xt[:], x[:, sl])
                nc.sync.dma_start(ct[:], cnt[:, sl])
                at = ap.tile([P, chunk], dt)
                # A = 2x + cnt
                nc.vector.scalar_tensor_tensor(
                    at[:], xt[:], 2.0, ct[:],
                    mybir.AluOpType.mult, mybir.AluOpType.add,
                )
                # mask = (A - 1) < 8, as int32 (CopyPredicated wants an
                # integer mask dtype; 32-bit keeps the 2x DVE perf mode)
                mt = mp.tile([P, chunk], mybir.dt.int32)
                nc.vector.tensor_scalar(
                    mt[:], at[:], 1.0, 8.0,
                    mybir.AluOpType.subtract, mybir.AluOpType.is_lt,
                )
                # u = 1 - x on the scalar engine
                ut = up.tile([P, chunk], dt)
                nc.scalar.activation(
                    ut[:], xt[:], mybir.ActivationFunctionType.Copy,
                    bias=1.0, scale=-1.0,
                )
                # x = where(mask, 1-x, x), in place; then store
                nc.vector.copy_predicated(xt[:], mt[:], ut[:])
                nc.sync.dma_start(out[:, sl], xt[:])
    nc.finalize()
    return nc


_NC_CACHE: dict[tuple, bass.Bass] = {}


def _get_nc(kind: str, fd: int, chunk: int) -> bass.Bass:
    key = (kind, fd, chunk)
    if key not in _NC_CACHE:
        build = build_kernel_pe if kind == "pe" else build_kernel_f32
        _NC_CACHE[key] = build(fd, chunk)
    return _NC_CACHE[key]


def _fast_path_ok(x: np.ndarray, c: np.ndarray) -> bool:
    # Fast path needs x to be 0/1 bits and c an integer counter in [0, 15]
    # (the FSUAbs operating domain); both are then exact in fp8_e4m3.
    if not np.all((x == 0.0) | (x == 1.0)):
        return False
    if not (c.min() >= 0.0 and c.max() <= 15.0):
        return False
    return bool(np.all(c == np.trunc(c)))


def _weight_matrix() -> np.ndarray:
    # lhsT[k, m]: U[m] = 16 * rhs[m] + 1 * rhs[m + 64]
    w = np.zeros((P, H), dtype=np.float32)
    w[np.arange(H), np.arange(H)] = 16.0
    w[np.arange(H) + H, np.arange(H)] = 1.0
    return w.astype(FP8)


def kernel(**inputs: np.ndarray) -> np.ndarray:
    global LAST_RESULTS
    x_full = np.ascontiguousarray(inputs["inp_Pr_i_j"], dtype=np.float32)
    c_full = np.ascontiguousarray(inputs["cnt_x"], dtype=np.float32)
    assert x_full.shape == FULL_SHAPE and c_full.shape == FULL_SHAPE

    fd = PER_CORE // P  # 16384
    fast = _fast_path_ok(x_full, c_full)
    if fast:
        nc = _get_nc("pe", fd, CHUNK)
        xr = x_full.astype(FP8).reshape(N_CORES, P, fd)
        cr = c_full.astype(FP8).reshape(N_CORES, P, fd)
        a1 = np.ascontiguousarray(
            np.concatenate([xr[:, :H], cr[:, :H]], axis=1)
        )
        a2 = np.ascontiguousarray(
            np.concatenate([xr[:, H:], cr[:, H:]], axis=1)
        )
        wm = _weight_matrix()
        in_maps = [{"a1": a1[c], "a2": a2[c], "w": wm} for c in range(N_CORES)]
    else:
        nc = _get_nc("f32", fd, CHUNK_F32)
        xs = x_full.reshape(N_CORES, P, fd)
        cs = c_full.reshape(N_CORES, P, fd)
        in_maps = [{"x": xs[c], "cnt": cs[c]} for c in range(N_CORES)]

    res = run_bass_kernel_spmd(
        nc, in_maps, list(range(N_CORES)), trace=TRACE, tmpdir=TMPDIR
    )
    LAST_RESULTS = res
    out = np.stack([res.results[c]["out"] for c in range(N_CORES)], axis=0)
    return np.ascontiguousarray(
        out.reshape(FULL_SHAPE).astype(np.float32)
    )


# revision 13
# speedup vs baseline: 2.3919x; 1.1571x over previous
"""Trainium2 Bass kernel for GainesEdgeDetect (single stochastic bit-cycle).

The reference module hardcodes sel=0 (first Sobol draw), so the MUXes
statically select their first operand and the output reduces to a pointwise
function of only inp_Pr_i_j (x) and cnt_x (c):

    A    = c + 2*x            (counter update, pre-clip)
    mask = (A - 1) < 8        (clip to [0,15] cannot change this comparison)
    out  = mask ? (1 - x) : x

Fast path (x is a 0/1 bitstream, c an integer counter in [0, 15] — the
module's actual operating domain): out == x XOR (2x + c < 9).  With
U = 16x + c (ranges of the two x-cases don't overlap), the truth table
collapses to a single band test:

    out = 1  iff  U <= 8  or  U >= 23   ==   |U - 15.5| > 7.0

Engine mapping, chosen from measured constraints (DVE two-tensor ops on
8-bit operands run at 1x = ~17.6us/core alone; DMA-casting fp8->bf16 paces
the SBUF fabric at the expanded byte count):

    PE : U = W.T @ moving           fp8 DoubleRow matmul (virtual K=256):
                                    W = [16*I | I], so one matmul computes
                                    all 128 partitions of 16x + c per bank
    ACT: A = Abs(U - 15.5)          straight from PSUM, writes bf16
    DVE: o = (A is_gt 7.0)          tensor_scalar, fp8 out (2x mode)

HBM traffic is cut 4x by keeping fp8_e4m3 on the wire (exact for 0/1 bits
and integer counters <= 15): 6 MiB/core instead of 24 MiB.  All DMAs are
plain HWDGE, sized >= 1 MiB.  The x/c planes are shipped block-interleaved
in 512-column blocks ([x-block | c-block] pairs), matching the DoubleRow
k-subtile layout, so each [128, 2, 512] rhs slice feeds one matmul.

Inputs that fail the domain check fall back to an exact f32 kernel.

Sharding: pointwise over 16M elements; each of the 8 cores takes a
contiguous 1/8th (2M elements) viewed as [128 partitions x 16384].
"""

import sys

for _p in ("/opt/trn_rl_repo", "/root/.axon_site/_ro/trn_rl_repo"):
    if _p not in sys.path:
        sys.path.append(_p)

import ml_dtypes
import numpy as np

import concourse.bacc as bacc
import concourse.bass as bass
import concourse.mybir as mybir
from concourse.bass_utils import run_bass_kernel_spmd
from concourse.tile import TileContext

N_CORES = 8
FULL_SHAPE = (16, 1024, 1024)
TOTAL = FULL_SHAPE[0] * FULL_SHAPE[1] * FULL_SHAPE[2]
PER_CORE = TOTAL // N_CORES  # 2M elements
P = 128  # SBUF partitions
H = P // 2  # half-stack rows
CHUNK = 2048  # fast-path tile free-dim ([128, 2048] f32 psum = 4 banks)
BANK = 512  # one PSUM bank holds 512 f32 per partition
CHUNK_F32 = 2048  # fallback tile free-dim

FP8 = ml_dtypes.float8_e4m3  # == mybir.dt.float8e4 wire format

# Set by test harness to capture an NTFF profile of the run.
TRACE = False
TMPDIR = None
LAST_RESULTS = None


NBL = 8  # 512-col blocks per load tile: [P, 8, 2, 512] fp8 = 1 MiB per DMA
PCH = 2048  # psum tile free-dim: [128, 2048] f32 = 4 banks, 2 tiles fill PSUM
OCH = 8192  # output store tile: [P, 8192] fp8 = 1 MiB per DMA


def build_kernel_pe(fd: int, chunk: int) -> bass.Bass:
    """Fast path: a[P, fd/512, 2, 512] fp8 x/c blocks -> out[P, fd] fp8."""
    nblk = fd // BANK
    assert nblk % NBL == 0 and (NBL * BANK) % PCH == 0 and OCH == 2 * NBL * BANK
    nc = bacc.Bacc()
    f8 = mybir.dt.float8e4
    bf = mybir.dt.bfloat16
    f32 = mybir.dt.float32
    a = nc.declare_dram_parameter("a", [P, nblk, 2, BANK], f8, isOutput=False)
    w = nc.declare_dram_parameter("w", [P, 2, P], f8, isOutput=False)
    out = nc.declare_dram_parameter("out", [P, fd], f8, isOutput=True)

    # ACT converts a float bias into a const AP; -15.5 isn't in the default
    # registry, so register it the same way Bass.__init__ does.
    bias_t = nc.alloc_sbuf_tensor("const-f32-m15p5", [P, 1], f32)
    nc.gpsimd.memset(bias_t.ap(), -15.5)
    nc.const_aps.aps[(f32, -15.5)] = bias_t.ap()
    nc.all_engine_barrier()

    with TileContext(nc) as tc:
        with (
            tc.tile_pool(name="wp", bufs=1) as wp,
            tc.tile_pool(name="rp", bufs=3) as rp,
            tc.tile_pool(name="ap", bufs=3) as ap,
            tc.tile_pool(name="op", bufs=2) as op,
            tc.tile_pool(name="pp", bufs=2, space="PSUM") as pp,
        ):
            wt = wp.tile([P, 2, P], f8)
            nc.sync.dma_start(wt[:], w[:, :, :])
            ot = None
            for i in range(nblk // NBL):
                rt = rp.tile([P, NBL, 2, BANK], f8)
                nc.sync.dma_start(rt[:], a[:, bass.ts(i, NBL)])
                if i % 2 == 0:
                    ot = op.tile([P, OCH], f8)
                for j in range((NBL * BANK) // PCH):
                    # U = 16x + c: one full-width DoubleRow matmul per bank
                    pt = pp.tile([P, PCH], f32)
                    for b in range(PCH // BANK):
                        nc.tensor.matmul(
                            pt[:, bass.ts(b, BANK)],
                            wt[:],
                            rt[:, j * (PCH // BANK) + b],
                            perf_mode=mybir.MatmulPerfMode.DoubleRow,
                        )
                    # A = |U - 15.5|, PSUM -> SBUF on the scalar engine
                    at = ap.tile([P, PCH], bf)
                    nc.scalar.activation(
                        at[:], pt[:], mybir.ActivationFunctionType.Abs,
                        bias=-15.5, scale=1.0,
                    )
                    # out = (A > 7.0), fp8, accumulated into the store tile
                    slot = (i % 2) * ((NBL * BANK) // PCH) + j
                    nc.vector.tensor_scalar(
                        ot[:, bass.ts(slot, PCH)], at[:], 7.0, None,
                        mybir.AluOpType.is_gt,
                    )
                if i % 2 == 1:
                    nc.sync.dma_start(out[:, bass.ts(i // 2, OCH)], ot[:])
    nc.finalize()
    return nc


def build_kernel_f32(fd: int, chunk: int) -> bass.Bass:
    """Fallback, exact for any f32 inputs: x[P, fd], cnt[P, fd] -> out[P, fd]."""
    assert fd % chunk == 0
    # Bacc (not plain Bass): its generate_event_semaphores pass splits
    # multi-sem waits into EventSemaphore instructions — TRN2 TPB compute
    # instructions can carry at most one sync-wait command.
    nc = bacc.Bacc()
    dt = mybir.dt.float32
    x = nc.declare_dram_parameter("x", [P, fd], dt, isOutput=False)
    cnt = nc.declare_dram_parameter("cnt", [P, fd], dt, isOutput=False)
    out = nc.declare_dram_parameter("out", [P, fd], dt, isOutput=True)

    with TileContext(nc) as tc:
        with (
            tc.tile_pool(name="xp", bufs=3) as xp,
            tc.tile_pool(name="cp", bufs=3) as cp,
            tc.tile_pool(name="ap", bufs=3) as ap,
            tc.tile_pool(name="mp", bufs=3) as mp,
            tc.tile_pool(name="up", bufs=3) as up,
        ):
            for i in range(fd // chunk):
                sl = bass.ts(i, chunk)
                xt = xp.tile([P, chunk], dt)
                ct = cp.tile([P, chunk], dt)
                nc.sync.dma_start(xt[:], x[:, sl])
                nc.sync.dma_start(ct[:], cnt[:, sl])
                at = ap.tile([P, chunk], dt)
                # A = 2x + cnt
                nc.vector.scalar_tensor_tensor(
                    at[:], xt[:], 2.0, ct[:],
                    mybir.AluOpType.mult, mybir.AluOpType.add,
                )
                # mask = (A - 1) < 8, as int32 (CopyPredicated wants an
                # integer mask dtype; 32-bit keeps the 2x DVE perf mode)
                mt = mp.tile([P, chunk], mybir.dt.int32)
                nc.vector.tensor_scalar(
                    mt[:], at[:], 1.0, 8.0,
                    mybir.AluOpType.subtract, mybir.AluOpType.is_lt,
                )
                # u = 1 - x on the scalar engine
                ut = up.tile([P, chunk], dt)
                nc.scalar.activation(
                    ut[:], xt[:], mybir.ActivationFunctionType.Copy,
                    bias=1.0, scale=-1.0,
                )
                # x = where(mask, 1-x, x), in place; then store
                nc.vector.copy_predicated(xt[:], mt[:], ut[:])
                nc.sync.dma_start(out[:, sl], xt[:])
    nc.finalize()
    return nc


_NC_CACHE: dict[tuple, bass.Bass] = {}


def _get_nc(kind: str, fd: int, chunk: int) -> bass.Bass:
    key = (kind, fd, chunk)
    if key not in _NC_CACHE:
        build = build_kernel_pe if kind == "pe" else build_kernel_f32
        _NC_CACHE[key] = build(fd, chunk)
    return _NC_CACHE[key]


def _fast_path_ok(x: np.ndarray, c: np.ndarray) -> bool:
    # Fast path needs x to be 0/1 bits and c an integer counter in [0, 15]
    # (the FSUAbs operating domain); both are then exact in fp8_e4m3.
    if not np.all((x == 0.0) | (x == 1.0)):
        return False
    if not (c.min() >= 0.0 and c.max() <= 15.0):
        return False
    return bool(np.all(c == np.trunc(c)))


def _weight_matrix() -> np.ndarray:
    # DoubleRow lhsT[k, i, m]: U[m] = 16 * rhs[k=m, i=0] + 1 * rhs[k=m, i=1]
    w = np.zeros((P, 2, P), dtype=np.float32)
    w[np.arange(P), 0, np.arange(P)] = 16.0
    w[np.arange(P), 1, np.arange(P)] = 1.0
    return w.astype(FP8)


def kernel(**inputs: np.ndarray) -> np.ndarray:
    global LAST_RESULTS
    x_full = np.ascontiguousarray(inputs["inp_Pr_i_j"], dtype=np.float32)
    c_full = np.ascontiguousarray(inputs["cnt_x"], dtype=np.float32)
    assert x_full.shape == FULL_SHAPE and c_full.shape == FULL_SHAPE

    fd = PER_CORE // P  # 16384
    fast = _fast_path_ok(x_full, c_full)
    if fast:
        nc = _get_nc("pe", fd, CHUNK)
        nblk = fd // BANK
        xr = x_full.astype(FP8).reshape(N_CORES, P, nblk, BANK)
        cr = c_full.astype(FP8).reshape(N_CORES, P, nblk, BANK)
        # block-interleave: a[:, :, blk] = [x-block | c-block], 512 cols each
        av = np.ascontiguousarray(np.stack([xr, cr], axis=3))
        wm = _weight_matrix()
        in_maps = [{"a": av[c], "w": wm} for c in range(N_CORES)]
    else:
        nc = _get_nc("f32", fd, CHUNK_F32)
        xs = x_full.reshape(N_CORES, P, fd)
        cs = c_full.reshape(N_CORES, P, fd)
        in_maps = [{"x": xs[c], "cnt": cs[c]} for c in range(N_CORES)]

    res = run_bass_kernel_spmd(
        nc, in_maps, list(range(N_CORES)), trace=TRACE, tmpdir=TMPDIR
    )
    LAST_RESULTS = res
    out = np.stack([res.results[c]["out"] for c in range(N_CORES)], axis=0)
    return np.ascontiguousarray(
        out.reshape(FULL_SHAPE).astype(np.float32)
    )
